# revision 1
# baseline (speedup 1.0000x reference)
"""Trainium2 Bass kernel for ConditionalHierarchicalCrossEntropyLoss.

Data-parallel: shard y_pred/y_true along batch across 8 NeuronCores;
replicate the tiny per-class table; sum the per-core partials on host.

The kernel runs at the per-core HBM roofline (~187 us = 67.1 MB at
358 GB/s): the two streaming loads are split across both HWDGE rings
(y_true on the ACT ring via nc.scalar, y_pred on the SP ring via
nc.sync), and y_true is triple-buffered because its consumer chain
(DVE max+max_index, ~17 us/block) is slower than its ~12.6 us load.

Per 128-row block on each core:
  1. DMA y_true block [128, 8192] -> SBUF. DVE InstMax gives the top-8
     values per row; InstMaxIndex over the full row gives the label
     (first-occurrence, matching jnp.argmax).
  2. gpsimd indirect DMA gathers the per-class record from a host-built
     DRAM table [C, 32]: paths 0..4, node masks [k<len], and
     wm_k = class_w * exp(-0.1*(len-1-k)) * [k < len-1].
  3. gpsimd indirect DMA gathers y_pred[row, path_k], k=0..4, from HBM
     using flat offsets row*8192 + path_k (one offset per partition per
     gather -- the HW DGE constraint). The k=5 entry is always the root
     (class 0) when it is unmasked, so it comes from column 0 of the
     exp'd y_pred tile instead of a 6th gather.
  4. DMA y_pred block -> SBUF; ACT exp-accumulate gives the softmax
     denominator Z per row (no max-subtraction: inputs ~ randn).
  5. Tiny [128, <=8] ops: suffix sums, conditional probs, ln, weighted
     row loss, accumulated across blocks.
Output per core: [128, 1] partials; host: loss = -sum(partials)/B.
"""

import numpy as np

import concourse.bacc as bacc
import concourse.bass as bass
import concourse.tile as tile
from concourse import mybir

N_CORES = 8
B = 8192          # batch
C = 8192          # classes
RPC = B // N_CORES  # rows per core
P = 128           # partitions / rows per block
NBLK = RPC // P   # blocks per core
D = 6             # max tree depth (padded path length)
NCHUNK = 64       # chunks per row for two-level argmax
CW = C // NCHUNK  # chunk width (128)
TW = 32           # table row width (floats)
EPS = 1e-8
DEPTH_PARAM = 0.1

f32 = mybir.dt.float32
u32 = mybir.dt.uint32

Alu = mybir.AluOpType
Act = mybir.ActivationFunctionType


WIN = 585   # tree mode: nodes at depth<=3 live in columns [0, WIN)


def _body(tc, yp_d, yt_d, tab_d, cst_d, cstu_d, out_d, dbg=None, repeats=1,
          tree_mode=False):
    NG = 2 if tree_mode else 5   # number of per-row random gathers
    # Software-pipelined by stage: engines execute in order, so per-block
    # chains with cross-engine round trips would stall every engine for all
    # later blocks. Emitting each stage for all 8 blocks together lets each
    # engine stream 8 homogeneous ops while SWDGE gather latencies amortize.
    nc = tc.nc
    with (
        tc.tile_pool(name="big", bufs=2) as big,
        tc.tile_pool(name="small", bufs=NBLK + 1) as small,
        tc.tile_pool(name="single", bufs=1) as single,
    ):
        cst = single.tile([P, 16], f32)
        nc.sync.dma_start(out=cst[:], in_=cst_d)
        cstu = single.tile([P, 12], u32)
        nc.sync.dma_start(out=cstu[:], in_=cstu_d)
        if tree_mode:
            iota_i = single.tile([P, WIN], mybir.dt.int32)
            nc.gpsimd.iota(iota_i[:], pattern=[[1, WIN]], base=0,
                           channel_multiplier=0)
            iota_f = single.tile([P, WIN], f32)
            nc.vector.tensor_copy(out=iota_f[:], in_=iota_i[:])

        acc = single.tile([P, 1], f32)

        for rep in range(repeats):
            nc.vector.memset(acc[:], 0.0)
            rowbase = [cst[:, 8 + b:9 + b] for b in range(NBLK)]
            st = {k: {} for k in ("cmax", "m8", "cidx", "offc_u", "z", "rz",
                                  "root_e", "chunk", "inner", "inner_f",
                                  "lab_u", "rec", "offu", "g", "eg", "probs",
                                  "sn", "rsn", "cond", "lc", "pl")}

            def stage_a(b):
                # big loads, full-row argmax of y_true, Z of y_pred
                rows = slice(b * P, (b + 1) * P)
                # split the two streaming loads across the two HWDGE rings
                # (SP ring for y_pred, ACT ring for y_true) so per-DMA fixed
                # costs overlap instead of serializing on one FIFO ring
                # yt gets 3 bufs: its consumer chain (DVE max+max_index,
                # ~17us/block) is slower than the 12.6us load, so 2 bufs
                # would stall the ACT ring on buffer release
                # the first two yt loads ride the SP ring instead: the ACT
                # ring then carries only 6 of the 8 yt loads and drains
                # early, so every block's argmax chain finishes under the
                # stream and the single-shot drain is just exp+gather+tail
                yt = big.tile([P, C], f32, tag="yt", name=f"yt{rep}_{b}",
                              bufs=3)
                yt_eng = nc.sync if b < 2 else nc.scalar
                yt_eng.dma_start(out=yt[:], in_=yt_d[rows, :])
                yp = big.tile([P, C], f32, tag="yp", name=f"yp{rep}_{b}")
                nc.sync.dma_start(out=yp[:], in_=yp_d[rows, :])
                z = st["z"][b] = small.tile([P, 1], f32, tag="z",
                                            name=f"z{b}")
                nc.scalar.activation(out=yp[:], in_=yp[:], func=Act.Exp,
                                     accum_out=z[:])
                # keep the exp'd low columns (root, and in tree mode all
                # nodes of depth<=3) so yp's big tile can be released early
                wn = WIN if tree_mode else 1
                wc = st["root_e"][b] = small.tile(
                    [P, wn], f32, tag="root_e", name=f"root_e{b}", bufs=5)
                nc.scalar.copy(wc[:], yp[:, 0:wn])
                m8 = st["m8"][b] = small.tile([P, 8], f32, tag="m8",
                                              name=f"m8_{b}")
                nc.vector.max(m8[:], yt[:])
                lab8 = st["lab_u"][b] = small.tile([P, 8], u32, tag="lab_u",
                                                   name=f"lab_u{b}")
                nc.vector.max_index(lab8[:], m8[:], yt[:])

            def stage_b(b):
                # record gather by label
                rec = st["rec"][b] = small.tile([P, TW], f32, tag="rec",
                                                name=f"rec{b}")
                nc.gpsimd.indirect_dma_start(
                    out=rec[:], out_offset=None, in_=tab_d,
                    in_offset=bass.IndirectOffsetOnAxis(
                        ap=st["lab_u"][b][:, 0:1], axis=0),
                )

            def stage_c(b):
                # value offsets; path-logit gathers; 1/Z
                offu = st["offu"][b] = small.tile(
                    [P, NG], u32, tag="offu", name=f"offu{b}")
                nc.vector.tensor_scalar(
                    out=offu[:], in0=st["rec"][b][:, 0:NG],
                    scalar1=rowbase[b], scalar2=None, op0=Alu.add,
                )
                g = st["g"][b] = small.tile([P, NG], f32, tag="g",
                                            name=f"g{b}")
                for k in range(NG):
                    nc.gpsimd.indirect_dma_start(
                        out=g[:, k:k + 1], out_offset=None, in_=yp_d,
                        in_offset=bass.IndirectOffsetOnAxis(
                            ap=offu[:, k:k + 1], axis=1),
                    )
                rz = st["rz"][b] = small.tile([P, 1], f32, tag="rz",
                                              name=f"rz{b}")
                nc.vector.reciprocal(rz[:], st["z"][b][:])

            def stage_e(b):
                # per-row loss tail
                rec, rz = st["rec"][b], st["rz"][b]
                eg = st["eg"][b] = small.tile([P, NG], f32, tag="eg",
                                              name=f"eg{b}")
                nc.scalar.activation(out=eg[:], in_=st["g"][b][:],
                                     func=Act.Exp)
                probs = st["probs"][b] = small.tile(
                    [P, D], f32, tag="probs", name=f"probs{b}")
                wc = st["root_e"][b]
                if tree_mode:
                    # exp'd values for depth<=3 nodes come from the SBUF
                    # window: e_k = sum_j [j == path_k] * exp(x_j)
                    epack = small.tile([P, 4], f32, tag="epack",
                                       name=f"epack{b}")
                    junk = small.tile([P, WIN], f32, tag="junk",
                                      name=f"junk{b}", bufs=2)
                    for i, (kcol, w) in enumerate(((2, WIN), (3, 73),
                                                   (4, 9))):
                        nc.vector.scalar_tensor_tensor(
                            out=junk[:, 0:w], in0=iota_f[:, 0:w],
                            scalar=rec[:, kcol:kcol + 1], in1=wc[:, 0:w],
                            op0=Alu.is_equal, op1=Alu.mult,
                            accum_out=epack[:, i:i + 1],
                        )
                    nc.vector.tensor_copy(out=epack[:, 3:4], in_=wc[:, 0:1])
                    nc.vector.scalar_tensor_tensor(
                        out=probs[:, 0:2], in0=eg[:], scalar=rz[:, 0:1],
                        in1=rec[:, 16:18], op0=Alu.mult, op1=Alu.mult,
                    )
                    nc.vector.scalar_tensor_tensor(
                        out=probs[:, 2:6], in0=epack[:], scalar=rz[:, 0:1],
                        in1=rec[:, 18:22], op0=Alu.mult, op1=Alu.mult,
                    )
                else:
                    nc.vector.scalar_tensor_tensor(
                        out=probs[:, 0:5], in0=eg[:], scalar=rz[:, 0:1],
                        in1=rec[:, 16:21], op0=Alu.mult, op1=Alu.mult,
                    )
                    nc.vector.scalar_tensor_tensor(
                        out=probs[:, 5:6], in0=wc[:], scalar=rz[:, 0:1],
                        in1=rec[:, 21:22], op0=Alu.mult, op1=Alu.mult,
                    )
                for k in range(D - 2, -1, -1):
                    nc.scalar.add(probs[:, k:k + 1], probs[:, k:k + 1],
                                  probs[:, k + 1:k + 2])
                sn = st["sn"][b] = small.tile([P, D - 1], f32, tag="sn",
                                              name=f"sn{b}")
                nc.scalar.activation(out=sn[:], in_=probs[:, 1:6],
                                     func=Act.Identity, bias=cst[:, 7:8])
                rsn = st["rsn"][b] = small.tile([P, D - 1], f32, tag="rsn",
                                                name=f"rsn{b}")
                nc.vector.reciprocal(rsn[:], sn[:])
                cond = st["cond"][b] = small.tile(
                    [P, D - 1], f32, tag="cond", name=f"cond{b}")
                nc.vector.tensor_tensor(out=cond[:], in0=probs[:, 0:5],
                                        in1=rsn[:], op=Alu.mult)
                lc = st["lc"][b] = small.tile([P, D - 1], f32, tag="lc",
                                              name=f"lc{b}")
                nc.scalar.activation(out=lc[:], in_=cond[:], func=Act.Ln,
                                     bias=cst[:, 7:8])
                t2 = small.tile([P, D - 1], f32, tag="t2", name=f"t2_{b}")
                pl = st["pl"][b] = small.tile([P, 1], f32, tag="pl",
                                              name=f"pl{b}")
                nc.vector.scalar_tensor_tensor(
                    out=t2[:], in0=lc[:], scalar=1.0, in1=rec[:, 8:13],
                    op0=Alu.mult, op1=Alu.mult, accum_out=pl[:],
                )
                nc.vector.tensor_tensor(out=acc[:], in0=acc[:], in1=pl[:],
                                        op=Alu.add)

            # software pipeline with block lag so each engine's in-order
            # stream interleaves stages of different blocks
            for s in range(NBLK + 3):
                if s < NBLK:
                    stage_a(s)
                if 0 <= s - 1 < NBLK:
                    stage_b(s - 1)
                if 0 <= s - 2 < NBLK:
                    stage_c(s - 2)
                if 0 <= s - 3 < NBLK:
                    stage_e(s - 3)

            if dbg is not None:
                lab_d, z_d, g_d, pl_d, off_d = dbg
                for b in range(NBLK):
                    rows = slice(b * P, (b + 1) * P)
                    labf = small.tile([P, 1], f32, tag="labf",
                                      name=f"labf{b}")
                    nc.vector.tensor_copy(out=labf[:],
                                          in_=st["lab_u"][b][:, 0:1])
                    nc.sync.dma_start(out=lab_d[rows, :], in_=labf[:])
                    nc.sync.dma_start(out=z_d[rows, :], in_=st["z"][b][:])
                    nc.sync.dma_start(out=g_d[rows, :], in_=st["g"][b][:])
                    nc.sync.dma_start(out=pl_d[rows, :], in_=st["pl"][b][:])
                    nc.sync.dma_start(out=off_d[rows, :],
                                      in_=st["offu"][b][:])

        nc.sync.dma_start(out=out_d, in_=acc[:])


def build_bass(debug_outs=False, repeats=1, tree_mode=False):
    nc = bacc.Bacc("TRN2", target_bir_lowering=False, debug=False,
                   enable_asserts=False)
    yp = nc.dram_tensor("y_pred_s", [RPC, C], f32, kind="ExternalInput")
    yt = nc.dram_tensor("y_true_s", [RPC, C], f32, kind="ExternalInput")
    tab = nc.dram_tensor("table", [C, TW], f32, kind="ExternalInput")
    cst = nc.dram_tensor("consts", [P, 16], f32, kind="ExternalInput")
    cstu = nc.dram_tensor("constsu", [P, 12], u32, kind="ExternalInput")
    out = nc.dram_tensor("partial", [P, 1], f32, kind="ExternalOutput")
    dbg = None
    if debug_outs:
        dbg = (
            nc.dram_tensor("lab_dbg", [RPC, 1], f32, kind="ExternalOutput").ap(),
            nc.dram_tensor("z_dbg", [RPC, 1], f32, kind="ExternalOutput").ap(),
            nc.dram_tensor("g_dbg", [RPC, D - 1], f32,
                           kind="ExternalOutput").ap(),
            nc.dram_tensor("pl_dbg", [RPC, 1], f32, kind="ExternalOutput").ap(),
            nc.dram_tensor("off_dbg", [RPC, D - 1], u32,
                           kind="ExternalOutput").ap(),
        )
    with tile.TileContext(nc) as tc:
        _body(tc, yp.ap(), yt.ap(), tab.ap(), cst.ap(), cstu.ap(), out.ap(),
              dbg, repeats=repeats, tree_mode=tree_mode)
    nc.compile()
    return nc


def make_host_tables(class_w, tree_paths, tree_lens):
    class_w = np.asarray(class_w, np.float64)
    lens = np.asarray(tree_lens, np.float64)
    table = np.zeros((C, TW), np.float32)
    table[:, 0:5] = np.asarray(tree_paths, np.float32)[:, 0:5]
    table[:, 6] = lens.astype(np.float32)
    k5 = np.arange(D - 1, dtype=np.float64)
    h = lens[:, None] - 1.0 - k5[None, :]
    w = np.exp(-DEPTH_PARAM * h.astype(np.float32).astype(np.float64))
    valid = k5[None, :] < (lens[:, None] - 1.0)
    table[:, 8:13] = (class_w[:, None] * w * valid).astype(np.float32)
    k6 = np.arange(D, dtype=np.float64)
    table[:, 16:22] = (k6[None, :] < lens[:, None]).astype(np.float32)

    consts = np.zeros((P, 16), np.float32)
    consts[:, 0:6] = np.arange(D, dtype=np.float32)[None, :]
    consts[:, 6] = 1.0
    consts[:, 7] = EPS
    p_idx = np.arange(P, dtype=np.float32)
    for b in range(NBLK):
        consts[:, 8 + b] = (b * P + p_idx) * C

    constsu = np.zeros((P, 12), np.uint32)
    for b in range(NBLK):
        constsu[:, b] = (b * P + np.arange(P, dtype=np.uint32)) * C
    constsu[:, 8] = CW
    return table, consts, constsu


def make_in_maps(y_pred, y_true, table, consts, constsu):
    y_pred = np.ascontiguousarray(np.asarray(y_pred, np.float32))
    y_true = np.ascontiguousarray(np.asarray(y_true, np.float32))
    in_maps = []
    for c in range(N_CORES):
        in_maps.append({
            "y_pred_s": y_pred[c * RPC:(c + 1) * RPC],
            "y_true_s": y_true[c * RPC:(c + 1) * RPC],
            "table": table,
            "consts": consts,
            "constsu": constsu,
        })
    return in_maps


_NC = {}


def tree_bounds_ok(tree_paths):
    p = np.asarray(tree_paths)
    return bool((p[:, 2].max() < WIN) and (p[:, 3].max() < 73)
                and (p[:, 4].max() < 9))


def kernel(y_pred, y_true, class_w, tree_paths, tree_lens):
    from concourse.bass_utils import run_bass_kernel_spmd
    tm = tree_bounds_ok(tree_paths)
    if tm not in _NC:
        _NC[tm] = build_bass(tree_mode=tm)
    _nc = _NC[tm]
    table, consts, constsu = make_host_tables(class_w, tree_paths, tree_lens)
    in_maps = make_in_maps(y_pred, y_true, table, consts, constsu)
    res = run_bass_kernel_spmd(_nc, in_maps, core_ids=list(range(N_CORES)))
    total = sum(float(r["partial"].sum()) for r in res.results)
    return np.float32(-total / B)


if __name__ == "__main__":
    nc = build_bass()
    print("built OK:", len(nc.m.functions[0].allocations), "allocations")



# revision 12
# speedup vs baseline: 1.5642x; 1.5642x over previous
"""Trainium2 Bass kernel for ConditionalHierarchicalCrossEntropyLoss.

Data-parallel: shard y_pred/y_true along batch across 8 NeuronCores;
replicate the tiny per-class table; sum the per-core partials on host.

The key identity: regrouping the per-level conditional log-probs as
sum_j c_j * ln(s_j + eps) with c_j = w'_j - w'_{j-1} (telescoped depth
weights) makes sum_j c_j = 0 exactly, so the softmax normalizer Z
cancels out of the loss. y_pred therefore never needs to be streamed:
only the exp'd low-column window (all tree nodes of depth<=3 live in
columns [0, 585)) and two gathered deep-path logits per row are
needed. Per-core HBM traffic is ~32 MB of y_true (full argmax scan)
plus ~3 MB of y_pred windows/gathers, which pins the kernel to the
~358 GB/s per-core HBM roofline at ~95-105 us.

Pipeline per 128-row block (y_true streamed as two half DMAs):
  A: DVE tensor_reduce(max) over [128, 32, 128] views gives 64 chunk
     maxima; tiny max/max_index find the winning chunk; gpsimd gathers
     that 128-wide chunk back from y_true in HBM.
  B: DVE max_index over the gathered chunk gives the within-chunk
     index (first-occurrence tie-break matches jnp.argmax exactly);
     label = chunk*128 + index; gpsimd gathers the per-class record
     (paths, masks, c_j coefficients) from a host-built table [C, 32].
  C: DVE computes flat offsets; gpsimd gathers the two deep path
     logits from y_pred in HBM; ACT exponentiates them (same table set
     as the window exps - no table switch).
  D: DVE-only tail: iota-is_equal dot products against the exp'd
     window extract the shallow path e-values; probs are assembled
     root-first so one tensor_tensor_scan gives all suffix sums.
Finals (once per rep): single Ln table switch, then ln(s_j+eps) and
the c-weighted accumulation for all 8 blocks.
Output per core: [128, 1] partials; host: loss = -sum(partials)/B.
"""

import numpy as np

import concourse.bacc as bacc
import concourse.bass as bass
import concourse.tile as tile
from concourse.bass import _add_dep_helper
from concourse import mybir

N_CORES = 8
B = 8192          # batch
C = 8192          # classes
RPC = B // N_CORES  # rows per core
P = 128           # partitions / rows per block
NBLK = RPC // P   # blocks per core
D = 6             # max tree depth (padded path length)
NCH = 64          # chunks per row for two-level argmax
CW = C // NCH     # chunk width (128)
HCH = NCH // 2    # chunks per half row
TW = 32           # table row width (floats)
EPS = 1e-8
DEPTH_PARAM = 0.1
HALF = C // 2
QTR = C // 4
QCH = NCH // 4    # chunks per quarter row

f32 = mybir.dt.float32
u32 = mybir.dt.uint32

Alu = mybir.AluOpType
Act = mybir.ActivationFunctionType
AxisX = mybir.AxisListType.X

WIN = 585   # tree mode: nodes at depth<=3 live in columns [0, WIN)
W3 = 73     # nodes at depth<=2
W4 = 9      # nodes at depth<=1

# pipeline tuning knobs (sweepable)
YTBUFS = 4   # y_true tile double-buffer depth
LAGB = 4     # steps between chunk-gather issue and its first use
LAGC = 8     # steps between record-gather issue and its first use
LAGD = 12    # steps between logit-gather issue and the DVE tail


def _body(tc, yp_d, yt_d, tab_d, cst_d, out_d, dbg=None, repeats=1,
          tree_mode=False):
    NG = 2 if tree_mode else 5   # per-row random y_pred gathers
    WN = WIN if tree_mode else 1
    nc = tc.nc
    ytr_d = yt_d.rearrange("a (b c) -> (a b) c", c=CW)   # [RPC*NCH, CW]
    with (
        tc.tile_pool(name="big", bufs=2) as big,
        tc.tile_pool(name="small", bufs=NBLK + 1) as small,
        tc.tile_pool(name="single", bufs=1) as single,
    ):
        cst = single.tile([P, 24], f32)
        nc.sync.dma_start(out=cst[:], in_=cst_d)
        # preload the combined exp+ln table set so the per-rep Exp->Ln
        # transition never costs a ~2.6us ACT table switch
        nc.scalar.add_instruction(mybir.InstLoadActFuncSet(
            name=nc.get_next_instruction_name(), ins=[], outs=[],
            act_func_set_id=6))
        if tree_mode:
            iota_i = single.tile([P, WIN], mybir.dt.int32)
            nc.gpsimd.iota(iota_i[:], pattern=[[1, WIN]], base=0,
                           channel_multiplier=0)
            iota_f = single.tile([P, WIN], f32)
            nc.vector.tensor_copy(out=iota_f[:], in_=iota_i[:])

        acc = single.tile([P, 1], f32)
        srall = single.tile([P, D * NBLK], f32)
        cjall = single.tile([P, D * NBLK], f32)
        trall = single.tile([P, D * NBLK], f32)

        last_i = {}
        for rep in range(repeats):
            rowbase = [cst[:, 8 + b:9 + b] for b in range(NBLK)]   # row*C f32
            rb64 = [cst[:, 16 + b:17 + b] for b in range(NBLK)]    # row*NCH
            st = {k: {} for k in ("yt", "wc", "cm", "m8", "ci8", "cif",
                                  "woff", "wch", "wi8", "wif", "lab", "rec",
                                  "offu", "g", "eg", "eg_i", "pr", "sr",
                                  "tr", "pl")}

            def em(key, inst):
                # pin per-engine instruction order: the Tile scheduler
                # otherwise reorders ops and lets gather-dependent ops
                # head-of-line block the in-order engine queues
                if inst is None or not hasattr(inst, "ins"):
                    return inst
                prev = last_i.get(key)
                if prev is not None:
                    _add_dep_helper(inst.ins, prev.ins, sync=False,
                                    reason="pin engine order")
                last_i[key] = inst
                return inst

            # y_pred exp windows: tiny, loaded upfront on the ACT ring so
            # the y_true stream owns the SP ring uninterrupted
            for b in range(NBLK):
                rows = slice(b * P, (b + 1) * P)
                wc = st["wc"][b] = small.tile([P, WN], f32, tag="wc",
                                              name=f"wc{b}")
                em("s", nc.scalar.dma_start(out=wc[:], in_=yp_d[rows, 0:WN]))
                em("s", nc.scalar.activation(out=wc[:], in_=wc[:],
                                             func=Act.Exp))

            def stage_a(b):
                # y_true half loads + chunk maxima + chunk argmax
                rows = slice(b * P, (b + 1) * P)
                yt = st["yt"][b] = big.tile([P, C], f32, tag="yt",
                                            name=f"yt{rep}_{b}", bufs=YTBUFS)
                for q in range(4):
                    em("q", nc.sync.dma_start(
                        out=yt[:, q * QTR:(q + 1) * QTR],
                        in_=yt_d[rows, q * QTR:(q + 1) * QTR]))
                cm = st["cm"][b] = small.tile([P, NCH], f32, tag="cm",
                                              name=f"cm{b}")
                for q in range(4):
                    em("v", nc.vector.tensor_reduce(
                        out=cm[:, q * QCH:(q + 1) * QCH],
                        in_=yt[:, q * QTR:(q + 1) * QTR].rearrange(
                            "p (c w) -> p c w", w=CW),
                        axis=AxisX, op=Alu.max))
                m8 = st["m8"][b] = small.tile([P, 8], f32, tag="m8",
                                              name=f"m8_{b}")
                em("v", nc.vector.max(m8[:], cm[:]))
                ci8 = st["ci8"][b] = small.tile([P, 8], u32, tag="ci8",
                                                name=f"ci8_{b}")
                em("v", nc.vector.max_index(ci8[:], m8[:], cm[:]))
                cif = st["cif"][b] = small.tile([P, 1], f32, tag="cif",
                                                name=f"cif{b}")
                em("v", nc.vector.tensor_copy(out=cif[:], in_=ci8[:, 0:1]))
                woff = st["woff"][b] = small.tile([P, 1], u32, tag="woff",
                                                  name=f"woff{b}")
                em("v", nc.vector.tensor_scalar(
                    out=woff[:], in0=cif[:], scalar1=rb64[b],
                    scalar2=None, op0=Alu.add))
                wch = st["wch"][b] = small.tile([P, CW], f32, tag="wch",
                                                name=f"wch{b}")
                em("p", nc.gpsimd.indirect_dma_start(
                    out=wch[:], out_offset=None, in_=ytr_d,
                    in_offset=bass.IndirectOffsetOnAxis(ap=woff[:, 0:1],
                                                        axis=0)))

            def stage_b(b):
                # within-chunk argmax -> label -> record gather
                wi8 = st["wi8"][b] = small.tile([P, 8], u32, tag="wi8",
                                                name=f"wi8_{b}")
                em("v", nc.vector.max_index(wi8[:], st["m8"][b][:],
                                            st["wch"][b][:]))
                wif = st["wif"][b] = small.tile([P, 1], f32, tag="wif",
                                                name=f"wif{b}")
                em("v", nc.vector.tensor_copy(out=wif[:], in_=wi8[:, 0:1]))
                lab = st["lab"][b] = small.tile([P, 1], u32, tag="lab",
                                                name=f"lab{b}")
                em("v", nc.vector.scalar_tensor_tensor(
                    out=lab[:], in0=st["cif"][b][:], scalar=float(CW),
                    in1=wif[:], op0=Alu.mult, op1=Alu.add))
                rec = st["rec"][b] = small.tile([P, TW], f32, tag="rec",
                                                name=f"rec{b}")
                em("p", nc.gpsimd.indirect_dma_start(
                    out=rec[:], out_offset=None, in_=tab_d,
                    in_offset=bass.IndirectOffsetOnAxis(ap=lab[:, 0:1],
                                                        axis=0)))

            def stage_c(b):
                # y_pred deep path-logit gathers + exp
                offu = st["offu"][b] = small.tile(
                    [P, NG], u32, tag="offu", name=f"offu{b}")
                em("v", nc.vector.tensor_scalar(
                    out=offu[:], in0=st["rec"][b][:, 0:NG],
                    scalar1=rowbase[b], scalar2=None, op0=Alu.add))
                g = st["g"][b] = small.tile([P, NG], f32, tag="g",
                                            name=f"g{b}")
                for k in range(NG):
                    em("p", nc.gpsimd.indirect_dma_start(
                        out=g[:, k:k + 1], out_offset=None, in_=yp_d,
                        in_offset=bass.IndirectOffsetOnAxis(
                            ap=offu[:, k:k + 1], axis=1),
                    ))
                eg = st["eg"][b] = small.tile([P, NG], f32, tag="eg",
                                              name=f"eg{b}")
                st["eg_i"][b] = em("s", nc.scalar.activation(
                    out=eg[:], in_=g[:], func=Act.Exp))

            def stage_d(b):
                # DVE-only: unnormalized masked e-values, root-first, and
                # their prefix scan (= suffix sums)
                rec = st["rec"][b]
                eg = st["eg"][b]
                pr = st["pr"][b] = small.tile([P, D], f32, tag="pr",
                                              name=f"pr{b}")
                wc = st["wc"][b]
                if tree_mode:
                    epk = small.tile([P, 4], f32, tag="epk", name=f"epk{b}")
                    junk = small.tile([P, WIN], f32, tag="junk",
                                      name=f"junk{b}", bufs=2)
                    em("v", nc.vector.tensor_copy(out=epk[:, 0:1],
                                                  in_=wc[:, 0:1]))
                    for i, (kcol, w) in enumerate(((4, W4), (3, W3),
                                                   (2, WIN))):
                        em("v", nc.vector.scalar_tensor_tensor(
                            out=junk[:, 0:w], in0=iota_f[:, 0:w],
                            scalar=rec[:, kcol:kcol + 1], in1=wc[:, 0:w],
                            op0=Alu.is_equal, op1=Alu.mult,
                            accum_out=epk[:, i + 1:i + 2],
                        ))
                    em("v", nc.vector.tensor_tensor(out=pr[:, 0:4],
                                                    in0=epk[:],
                                                    in1=rec[:, 16:20],
                                                    op=Alu.mult))
                    em("v", nc.vector.tensor_tensor(out=pr[:, 4:6],
                                                    in0=eg[:],
                                                    in1=rec[:, 20:22],
                                                    op=Alu.mult))
                else:
                    em("v", nc.vector.tensor_tensor(out=pr[:, 0:1],
                                                    in0=wc[:],
                                                    in1=rec[:, 16:17],
                                                    op=Alu.mult))
                    em("v", nc.vector.tensor_tensor(out=pr[:, 1:6],
                                                    in0=eg[:],
                                                    in1=rec[:, 17:22],
                                                    op=Alu.mult))
                em("v", nc.vector.tensor_tensor_scan(
                    out=srall[:, D * b:D * (b + 1)], data0=pr[:],
                    data1=pr[:], initial=0.0,
                    op0=Alu.add, op1=Alu.bypass))
                em("v", nc.vector.tensor_copy(
                    out=cjall[:, D * b:D * (b + 1)], in_=rec[:, 8:14]))

            # software pipeline, deepest stage first inside each step so
            # gather-dependent ops never head-of-line block the in-order
            # engine queues; four steps of lag per gather round-trip so
            # gather latency (queued behind up to a full buffer depth of
            # in-flight stream DMAs) never stalls DVE
            for s in range(NBLK + LAGD):
                if 0 <= s - LAGD < NBLK:
                    stage_d(s - LAGD)
                if 0 <= s - LAGC < NBLK:
                    stage_c(s - LAGC)
                if 0 <= s - LAGB < NBLK:
                    stage_b(s - LAGB)
                if s < NBLK:
                    stage_a(s)

            # finals: one Ln over all blocks' suffix sums and one
            # c-weighted accumulate; acc is fully overwritten per rep
            em("s", nc.scalar.activation(
                out=trall[:], in_=srall[:], func=Act.Ln,
                bias=cst[:, 7:8]))
            em("v", nc.vector.scalar_tensor_tensor(
                out=trall[:], in0=trall[:], scalar=1.0, in1=cjall[:],
                op0=Alu.mult, op1=Alu.mult, accum_out=acc[:],
            ))

            if dbg is not None:
                lab_d, g_d, pl_d, off_d = dbg
                for b in range(NBLK):
                    rows = slice(b * P, (b + 1) * P)
                    labf = small.tile([P, 1], f32, tag="labf",
                                      name=f"labf{b}")
                    nc.vector.tensor_copy(out=labf[:],
                                          in_=st["lab"][b][:, 0:1])
                    nc.sync.dma_start(out=lab_d[rows, :], in_=labf[:])
                    nc.sync.dma_start(out=g_d[rows, :], in_=st["g"][b][:])
                    nc.sync.dma_start(out=pl_d[rows, :], in_=st["pl"][b][:])
                    nc.sync.dma_start(out=off_d[rows, :],
                                      in_=st["offu"][b][:])

        nc.sync.dma_start(out=out_d, in_=acc[:])


def build_bass(debug_outs=False, repeats=1, tree_mode=False):
    nc = bacc.Bacc("TRN2", target_bir_lowering=False, debug=False,
                   enable_asserts=False)
    yp = nc.dram_tensor("y_pred_s", [RPC, C], f32, kind="ExternalInput")
    yt = nc.dram_tensor("y_true_s", [RPC, C], f32, kind="ExternalInput")
    tab = nc.dram_tensor("table", [C, TW], f32, kind="ExternalInput")
    cst = nc.dram_tensor("consts", [P, 24], f32, kind="ExternalInput")
    out = nc.dram_tensor("partial", [P, 1], f32, kind="ExternalOutput")
    dbg = None
    if debug_outs:
        dbg = (
            nc.dram_tensor("lab_dbg", [RPC, 1], f32, kind="ExternalOutput").ap(),
            nc.dram_tensor("g_dbg", [RPC, D - 1], f32,
                           kind="ExternalOutput").ap(),
            nc.dram_tensor("pl_dbg", [RPC, 1], f32, kind="ExternalOutput").ap(),
            nc.dram_tensor("off_dbg", [RPC, D - 1], u32,
                           kind="ExternalOutput").ap(),
        )
    with tile.TileContext(nc) as tc:
        _body(tc, yp.ap(), yt.ap(), tab.ap(), cst.ap(), out.ap(),
              dbg, repeats=repeats, tree_mode=tree_mode)
    nc.compile()
    return nc


def make_host_tables(class_w, tree_paths, tree_lens, tree_mode=True):
    class_w = np.asarray(class_w, np.float64)
    paths = np.asarray(tree_paths, np.float64)
    lens = np.asarray(tree_lens, np.float64)
    table = np.zeros((C, TW), np.float32)
    if tree_mode:
        # cols 0:2 = (path[1], path[0]) for the two HBM gathers (feed
        # pr[4], pr[5]); cols 2:5 = path[2..4] for the is_equal windows
        table[:, 0] = paths[:, 1]
        table[:, 1] = paths[:, 0]
        table[:, 2:5] = paths[:, 2:5]
    else:
        # five HBM gathers feed pr[1:6] root-first: (p4, p3, p2, p1, p0)
        table[:, 0:5] = paths[:, 4::-1]
    table[:, 6] = lens
    # depth weights w'_k = class_w * exp(-dp*(len-1-k)) * [k < len-1],
    # regrouped per ln(s_j): c_j = w'_j - w'_{j-1}; stored root-first
    # (col 8+j = c_{D-1-j}); sum_j c_j = 0, which cancels the softmax
    # normalizer Z out of the loss
    kk = np.arange(D, dtype=np.float64)
    h = lens[:, None] - 1.0 - kk[None, :]
    w = np.exp(-DEPTH_PARAM * h.astype(np.float32).astype(np.float64))
    valid = kk[None, :] < (lens[:, None] - 1.0)
    wp = class_w[:, None] * w * valid                       # w'_k, k=0..5
    cjs = wp.copy()
    cjs[:, 1:] -= wp[:, :-1]                                # c_j
    table[:, 8:14] = cjs[:, ::-1].astype(np.float32)        # root-first
    # node masks [k < len], root-first: col 16+j = [D-1-j < len]
    node_mask = (kk[None, :] < lens[:, None]).astype(np.float32)
    table[:, 16:22] = node_mask[:, ::-1]

    consts = np.zeros((P, 24), np.float32)
    consts[:, 6] = 1.0
    consts[:, 7] = EPS
    p_idx = np.arange(P, dtype=np.float32)
    for b in range(NBLK):
        consts[:, 8 + b] = (b * P + p_idx) * C
        consts[:, 16 + b] = (b * P + p_idx) * NCH
    return table, consts


def make_in_maps(y_pred, y_true, table, consts):
    y_pred = np.ascontiguousarray(np.asarray(y_pred, np.float32))
    y_true = np.ascontiguousarray(np.asarray(y_true, np.float32))
    in_maps = []
    for c in range(N_CORES):
        in_maps.append({
            "y_pred_s": y_pred[c * RPC:(c + 1) * RPC],
            "y_true_s": y_true[c * RPC:(c + 1) * RPC],
            "table": table,
            "consts": consts,
        })
    return in_maps


_NC = {}


def tree_bounds_ok(tree_paths):
    p = np.asarray(tree_paths)
    return bool((p[:, 2].max() < WIN) and (p[:, 3].max() < W3)
                and (p[:, 4].max() < W4))


def kernel(y_pred, y_true, class_w, tree_paths, tree_lens):
    from concourse.bass_utils import run_bass_kernel_spmd
    tm = tree_bounds_ok(tree_paths)
    if tm not in _NC:
        _NC[tm] = build_bass(tree_mode=tm)
    _nc = _NC[tm]
    table, consts = make_host_tables(class_w, tree_paths, tree_lens,
                                     tree_mode=tm)
    in_maps = make_in_maps(y_pred, y_true, table, consts)
    res = run_bass_kernel_spmd(_nc, in_maps, core_ids=list(range(N_CORES)))
    total = sum(float(r["partial"].sum()) for r in res.results)
    return np.float32(-total / B)


if __name__ == "__main__":
    nc = build_bass()
    print("built OK:", len(nc.m.functions[0].allocations), "allocations")


# revision 14
# speedup vs baseline: 1.8279x; 1.1686x over previous
"""Trainium2 Bass kernel for ConditionalHierarchicalCrossEntropyLoss.

Data-parallel: shard y_pred/y_true along batch across 8 NeuronCores;
replicate the tiny per-class table; sum the per-core partials on host.

The key identity: regrouping the per-level conditional log-probs as
sum_j c_j * ln(s_j + eps) with c_j = w'_j - w'_{j-1} (telescoped depth
weights) makes sum_j c_j = 0 exactly, so the softmax normalizer Z
cancels out of the loss. y_pred therefore never needs to be streamed:
only the exp'd low-column window (all tree nodes of depth<=3 live in
columns [0, 585)) and two gathered deep-path logits per row are
needed. Per-core HBM traffic is ~32 MB of y_true (full argmax scan)
plus ~3 MB of y_pred windows/gathers, which pins the kernel to the
~358 GB/s per-core HBM roofline at ~95-105 us.

Pipeline per 128-row block (y_true streamed as two half DMAs):
  A: DVE tensor_reduce(max) over [128, 32, 128] views gives 64 chunk
     maxima; tiny max/max_index find the winning chunk; gpsimd gathers
     that 128-wide chunk back from y_true in HBM.
  B: DVE max_index over the gathered chunk gives the within-chunk
     index (first-occurrence tie-break matches jnp.argmax exactly);
     label = chunk*128 + index; gpsimd gathers the per-class record
     (paths, masks, c_j coefficients) from a host-built table [C, 32].
  C: DVE computes flat offsets; gpsimd gathers the two deep path
     logits from y_pred in HBM; ACT exponentiates them (same table set
     as the window exps - no table switch).
  D: DVE-only tail: iota-is_equal dot products against the exp'd
     window extract the shallow path e-values; probs are assembled
     root-first so one tensor_tensor_scan gives all suffix sums.
Finals (once per rep): single Ln table switch, then ln(s_j+eps) and
the c-weighted accumulation for all 8 blocks.
Output per core: [128, 1] partials; host: loss = -sum(partials)/B.
"""

import numpy as np

import concourse.bacc as bacc
import concourse.bass as bass
import concourse.tile as tile
from concourse.bass import _add_dep_helper
from concourse import mybir

N_CORES = 8
B = 8192          # batch
C = 8192          # classes
RPC = B // N_CORES  # rows per core
P = 128           # partitions / rows per block
NBLK = RPC // P   # blocks per core
D = 6             # max tree depth (padded path length)
NCH = 64          # chunks per row for two-level argmax
CW = C // NCH     # chunk width (128)
HCH = NCH // 2    # chunks per half row
TW = 32           # table row width (floats)
EPS = 1e-8
DEPTH_PARAM = 0.1
HALF = C // 2
QTR = C // 4
QCH = NCH // 4    # chunks per quarter row

f32 = mybir.dt.float32
u32 = mybir.dt.uint32

Alu = mybir.AluOpType
Act = mybir.ActivationFunctionType
AxisX = mybir.AxisListType.X

WIN = 585   # tree mode: nodes at depth<=3 live in columns [0, WIN)
W3 = 73     # nodes at depth<=2
W4 = 9      # nodes at depth<=1

# pipeline tuning knobs (HW-tuned via loop-marginal A/B)
YTBUFS = 3   # y_true tile double-buffer depth
LAGB = 2     # steps between chunk-gather issue and its first use
LAGC = 4     # steps between record-gather issue and its first use
LAGD = 6     # steps between logit-gather issue and the DVE tail
QSPLIT = 4   # stream DMAs per y_true block (4=quarters, 2=halves)


def _body(tc, yp_d, yt_d, tab_d, cst_d, out_d, dbg=None, repeats=1,
          tree_mode=False, stream_only=False):
    NG = 2 if tree_mode else 5   # per-row random y_pred gathers
    WN = WIN if tree_mode else 1
    nc = tc.nc
    ytr_d = yt_d.rearrange("a (b c) -> (a b) c", c=CW)   # [RPC*NCH, CW]
    with (
        tc.tile_pool(name="big", bufs=2) as big,
        tc.tile_pool(name="small", bufs=NBLK + 1) as small,
        tc.tile_pool(name="single", bufs=1) as single,
    ):
        cst = single.tile([P, 24], f32)
        nc.sync.dma_start(out=cst[:], in_=cst_d)
        # preload the combined exp+ln table set so the per-rep Exp->Ln
        # transition never costs a ~2.6us ACT table switch
        nc.scalar.add_instruction(mybir.InstLoadActFuncSet(
            name=nc.get_next_instruction_name(), ins=[], outs=[],
            act_func_set_id=6))
        if tree_mode:
            iota_i = single.tile([P, WIN], mybir.dt.int32)
            nc.gpsimd.iota(iota_i[:], pattern=[[1, WIN]], base=0,
                           channel_multiplier=0)
            iota_f = single.tile([P, WIN], f32)
            nc.vector.tensor_copy(out=iota_f[:], in_=iota_i[:])

        acc = single.tile([P, 1], f32)
        srall = single.tile([P, D * NBLK], f32)
        cjall = single.tile([P, D * NBLK], f32)
        trall = single.tile([P, D * NBLK], f32)

        last_i = {}
        for rep in range(repeats):
            rowbase = [cst[:, 8 + b:9 + b] for b in range(NBLK)]   # row*C f32
            rb64 = [cst[:, 16 + b:17 + b] for b in range(NBLK)]    # row*NCH
            st = {k: {} for k in ("yt", "wc", "cm", "m8", "ci8", "cif",
                                  "woff", "wch", "wi8", "wif", "lab", "rec",
                                  "offu", "g", "eg", "eg_i", "pr", "sr",
                                  "tr", "pl")}

            def em(key, inst):
                # pin per-engine instruction order: the Tile scheduler
                # otherwise reorders ops and lets gather-dependent ops
                # head-of-line block the in-order engine queues
                if inst is None or not hasattr(inst, "ins"):
                    return inst
                prev = last_i.get(key)
                if prev is not None:
                    _add_dep_helper(inst.ins, prev.ins, sync=False,
                                    reason="pin engine order")
                last_i[key] = inst
                return inst

            # y_pred exp windows: tiny, loaded upfront on the ACT ring so
            # the y_true stream owns the SP ring uninterrupted
            for b in range(NBLK):
                rows = slice(b * P, (b + 1) * P)
                wc = st["wc"][b] = small.tile([P, WN], f32, tag="wc",
                                              name=f"wc{b}")
                em("s", nc.scalar.dma_start(out=wc[:], in_=yp_d[rows, 0:WN]))
                em("s", nc.scalar.activation(out=wc[:], in_=wc[:],
                                             func=Act.Exp))

            def stage_a(b):
                # y_true half loads + chunk maxima + chunk argmax
                rows = slice(b * P, (b + 1) * P)
                yt = st["yt"][b] = big.tile([P, C], f32, tag="yt",
                                            name=f"yt{rep}_{b}", bufs=YTBUFS)
                qw = C // QSPLIT
                qc = NCH // QSPLIT
                for q in range(QSPLIT):
                    em("q", nc.sync.dma_start(
                        out=yt[:, q * qw:(q + 1) * qw],
                        in_=yt_d[rows, q * qw:(q + 1) * qw]))
                cm = st["cm"][b] = small.tile([P, NCH], f32, tag="cm",
                                              name=f"cm{b}")
                for q in range(QSPLIT):
                    em("v", nc.vector.tensor_reduce(
                        out=cm[:, q * qc:(q + 1) * qc],
                        in_=yt[:, q * qw:(q + 1) * qw].rearrange(
                            "p (c w) -> p c w", w=CW),
                        axis=AxisX, op=Alu.max))
                m8 = st["m8"][b] = small.tile([P, 8], f32, tag="m8",
                                              name=f"m8_{b}")
                em("v", nc.vector.max(m8[:], cm[:]))
                ci8 = st["ci8"][b] = small.tile([P, 8], u32, tag="ci8",
                                                name=f"ci8_{b}")
                em("v", nc.vector.max_index(ci8[:], m8[:], cm[:]))
                cif = st["cif"][b] = small.tile([P, 1], f32, tag="cif",
                                                name=f"cif{b}")
                em("v", nc.vector.tensor_copy(out=cif[:], in_=ci8[:, 0:1]))
                woff = st["woff"][b] = small.tile([P, 1], u32, tag="woff",
                                                  name=f"woff{b}")
                em("v", nc.vector.tensor_scalar(
                    out=woff[:], in0=cif[:], scalar1=rb64[b],
                    scalar2=None, op0=Alu.add))
                wch = st["wch"][b] = small.tile([P, CW], f32, tag="wch",
                                                name=f"wch{b}")
                em("p", nc.gpsimd.indirect_dma_start(
                    out=wch[:], out_offset=None, in_=ytr_d,
                    in_offset=bass.IndirectOffsetOnAxis(ap=woff[:, 0:1],
                                                        axis=0)))

            def stage_b(b):
                # within-chunk argmax -> label -> record gather
                wi8 = st["wi8"][b] = small.tile([P, 8], u32, tag="wi8",
                                                name=f"wi8_{b}")
                em("v", nc.vector.max_index(wi8[:], st["m8"][b][:],
                                            st["wch"][b][:]))
                wif = st["wif"][b] = small.tile([P, 1], f32, tag="wif",
                                                name=f"wif{b}")
                em("v", nc.vector.tensor_copy(out=wif[:], in_=wi8[:, 0:1]))
                lab = st["lab"][b] = small.tile([P, 1], u32, tag="lab",
                                                name=f"lab{b}")
                em("v", nc.vector.scalar_tensor_tensor(
                    out=lab[:], in0=st["cif"][b][:], scalar=float(CW),
                    in1=wif[:], op0=Alu.mult, op1=Alu.add))
                rec = st["rec"][b] = small.tile([P, TW], f32, tag="rec",
                                                name=f"rec{b}")
                em("p", nc.gpsimd.indirect_dma_start(
                    out=rec[:], out_offset=None, in_=tab_d,
                    in_offset=bass.IndirectOffsetOnAxis(ap=lab[:, 0:1],
                                                        axis=0)))

            def stage_c(b):
                # y_pred deep path-logit gathers + exp
                offu = st["offu"][b] = small.tile(
                    [P, NG], u32, tag="offu", name=f"offu{b}")
                em("v", nc.vector.tensor_scalar(
                    out=offu[:], in0=st["rec"][b][:, 0:NG],
                    scalar1=rowbase[b], scalar2=None, op0=Alu.add))
                g = st["g"][b] = small.tile([P, NG], f32, tag="g",
                                            name=f"g{b}")
                for k in range(NG):
                    em("p", nc.gpsimd.indirect_dma_start(
                        out=g[:, k:k + 1], out_offset=None, in_=yp_d,
                        in_offset=bass.IndirectOffsetOnAxis(
                            ap=offu[:, k:k + 1], axis=1),
                    ))
                eg = st["eg"][b] = small.tile([P, NG], f32, tag="eg",
                                              name=f"eg{b}")
                st["eg_i"][b] = em("s", nc.scalar.activation(
                    out=eg[:], in_=g[:], func=Act.Exp))

            def stage_d(b):
                # DVE-only: unnormalized masked e-values, root-first, and
                # their prefix scan (= suffix sums)
                rec = st["rec"][b]
                eg = st["eg"][b]
                pr = st["pr"][b] = small.tile([P, D], f32, tag="pr",
                                              name=f"pr{b}")
                wc = st["wc"][b]
                if tree_mode:
                    epk = small.tile([P, 4], f32, tag="epk", name=f"epk{b}")
                    junk = small.tile([P, WIN], f32, tag="junk",
                                      name=f"junk{b}", bufs=2)
                    em("v", nc.vector.tensor_copy(out=epk[:, 0:1],
                                                  in_=wc[:, 0:1]))
                    for i, (kcol, w) in enumerate(((4, W4), (3, W3),
                                                   (2, WIN))):
                        em("v", nc.vector.scalar_tensor_tensor(
                            out=junk[:, 0:w], in0=iota_f[:, 0:w],
                            scalar=rec[:, kcol:kcol + 1], in1=wc[:, 0:w],
                            op0=Alu.is_equal, op1=Alu.mult,
                            accum_out=epk[:, i + 1:i + 2],
                        ))
                    em("v", nc.vector.tensor_tensor(out=pr[:, 0:4],
                                                    in0=epk[:],
                                                    in1=rec[:, 16:20],
                                                    op=Alu.mult))
                    em("v", nc.vector.tensor_tensor(out=pr[:, 4:6],
                                                    in0=eg[:],
                                                    in1=rec[:, 20:22],
                                                    op=Alu.mult))
                else:
                    em("v", nc.vector.tensor_tensor(out=pr[:, 0:1],
                                                    in0=wc[:],
                                                    in1=rec[:, 16:17],
                                                    op=Alu.mult))
                    em("v", nc.vector.tensor_tensor(out=pr[:, 1:6],
                                                    in0=eg[:],
                                                    in1=rec[:, 17:22],
                                                    op=Alu.mult))
                em("v", nc.vector.tensor_tensor_scan(
                    out=srall[:, D * b:D * (b + 1)], data0=pr[:],
                    data1=pr[:], initial=0.0,
                    op0=Alu.add, op1=Alu.bypass))
                em("v", nc.vector.tensor_copy(
                    out=cjall[:, D * b:D * (b + 1)], in_=rec[:, 8:14]))

            # software pipeline, deepest stage first inside each step so
            # gather-dependent ops never head-of-line block the in-order
            # engine queues; four steps of lag per gather round-trip so
            # gather latency (queued behind up to a full buffer depth of
            # in-flight stream DMAs) never stalls DVE
            if stream_only:
                for s in range(NBLK):
                    stage_a(s)
                for b in range(NBLK):
                    em("v", nc.vector.tensor_copy(
                        out=srall[:, D * b:D * (b + 1)],
                        in_=st["m8"][b][:, 0:D]))
                em("v", nc.vector.scalar_tensor_tensor(
                    out=trall[:], in0=srall[:], scalar=1.0, in1=srall[:],
                    op0=Alu.mult, op1=Alu.mult, accum_out=acc[:]))
                continue
            for s in range(NBLK + LAGD):
                if 0 <= s - LAGD < NBLK:
                    stage_d(s - LAGD)
                if 0 <= s - LAGC < NBLK:
                    stage_c(s - LAGC)
                if 0 <= s - LAGB < NBLK:
                    stage_b(s - LAGB)
                if s < NBLK:
                    stage_a(s)

            # finals: one Ln over all blocks' suffix sums and one
            # c-weighted accumulate; acc is fully overwritten per rep
            em("s", nc.scalar.activation(
                out=trall[:], in_=srall[:], func=Act.Ln,
                bias=cst[:, 7:8]))
            em("v", nc.vector.scalar_tensor_tensor(
                out=trall[:], in0=trall[:], scalar=1.0, in1=cjall[:],
                op0=Alu.mult, op1=Alu.mult, accum_out=acc[:],
            ))

            if dbg is not None:
                lab_d, g_d, pl_d, off_d = dbg
                for b in range(NBLK):
                    rows = slice(b * P, (b + 1) * P)
                    labf = small.tile([P, 1], f32, tag="labf",
                                      name=f"labf{b}")
                    nc.vector.tensor_copy(out=labf[:],
                                          in_=st["lab"][b][:, 0:1])
                    nc.sync.dma_start(out=lab_d[rows, :], in_=labf[:])
                    nc.sync.dma_start(out=g_d[rows, :], in_=st["g"][b][:])
                    nc.sync.dma_start(out=pl_d[rows, :], in_=st["pl"][b][:])
                    nc.sync.dma_start(out=off_d[rows, :],
                                      in_=st["offu"][b][:])

        nc.sync.dma_start(out=out_d, in_=acc[:])


def build_bass(debug_outs=False, repeats=1, tree_mode=False,
               stream_only=False):
    nc = bacc.Bacc("TRN2", target_bir_lowering=False, debug=False,
                   enable_asserts=False)
    yp = nc.dram_tensor("y_pred_s", [RPC, C], f32, kind="ExternalInput")
    yt = nc.dram_tensor("y_true_s", [RPC, C], f32, kind="ExternalInput")
    tab = nc.dram_tensor("table", [C, TW], f32, kind="ExternalInput")
    cst = nc.dram_tensor("consts", [P, 24], f32, kind="ExternalInput")
    out = nc.dram_tensor("partial", [P, 1], f32, kind="ExternalOutput")
    dbg = None
    if debug_outs:
        dbg = (
            nc.dram_tensor("lab_dbg", [RPC, 1], f32, kind="ExternalOutput").ap(),
            nc.dram_tensor("g_dbg", [RPC, D - 1], f32,
                           kind="ExternalOutput").ap(),
            nc.dram_tensor("pl_dbg", [RPC, 1], f32, kind="ExternalOutput").ap(),
            nc.dram_tensor("off_dbg", [RPC, D - 1], u32,
                           kind="ExternalOutput").ap(),
        )
    with tile.TileContext(nc) as tc:
        _body(tc, yp.ap(), yt.ap(), tab.ap(), cst.ap(), out.ap(),
              dbg, repeats=repeats, tree_mode=tree_mode,
              stream_only=stream_only)
    nc.compile()
    return nc


def make_host_tables(class_w, tree_paths, tree_lens, tree_mode=True):
    class_w = np.asarray(class_w, np.float64)
    paths = np.asarray(tree_paths, np.float64)
    lens = np.asarray(tree_lens, np.float64)
    table = np.zeros((C, TW), np.float32)
    if tree_mode:
        # cols 0:2 = (path[1], path[0]) for the two HBM gathers (feed
        # pr[4], pr[5]); cols 2:5 = path[2..4] for the is_equal windows
        table[:, 0] = paths[:, 1]
        table[:, 1] = paths[:, 0]
        table[:, 2:5] = paths[:, 2:5]
    else:
        # five HBM gathers feed pr[1:6] root-first: (p4, p3, p2, p1, p0)
        table[:, 0:5] = paths[:, 4::-1]
    table[:, 6] = lens
    # depth weights w'_k = class_w * exp(-dp*(len-1-k)) * [k < len-1],
    # regrouped per ln(s_j): c_j = w'_j - w'_{j-1}; stored root-first
    # (col 8+j = c_{D-1-j}); sum_j c_j = 0, which cancels the softmax
    # normalizer Z out of the loss
    kk = np.arange(D, dtype=np.float64)
    h = lens[:, None] - 1.0 - kk[None, :]
    w = np.exp(-DEPTH_PARAM * h.astype(np.float32).astype(np.float64))
    valid = kk[None, :] < (lens[:, None] - 1.0)
    wp = class_w[:, None] * w * valid                       # w'_k, k=0..5
    cjs = wp.copy()
    cjs[:, 1:] -= wp[:, :-1]                                # c_j
    table[:, 8:14] = cjs[:, ::-1].astype(np.float32)        # root-first
    # node masks [k < len], root-first: col 16+j = [D-1-j < len]
    node_mask = (kk[None, :] < lens[:, None]).astype(np.float32)
    table[:, 16:22] = node_mask[:, ::-1]

    consts = np.zeros((P, 24), np.float32)
    consts[:, 6] = 1.0
    consts[:, 7] = EPS
    p_idx = np.arange(P, dtype=np.float32)
    for b in range(NBLK):
        consts[:, 8 + b] = (b * P + p_idx) * C
        consts[:, 16 + b] = (b * P + p_idx) * NCH
    return table, consts


def make_in_maps(y_pred, y_true, table, consts):
    y_pred = np.ascontiguousarray(np.asarray(y_pred, np.float32))
    y_true = np.ascontiguousarray(np.asarray(y_true, np.float32))
    in_maps = []
    for c in range(N_CORES):
        in_maps.append({
            "y_pred_s": y_pred[c * RPC:(c + 1) * RPC],
            "y_true_s": y_true[c * RPC:(c + 1) * RPC],
            "table": table,
            "consts": consts,
        })
    return in_maps


_NC = {}


def tree_bounds_ok(tree_paths):
    p = np.asarray(tree_paths)
    return bool((p[:, 2].max() < WIN) and (p[:, 3].max() < W3)
                and (p[:, 4].max() < W4))


def kernel(y_pred, y_true, class_w, tree_paths, tree_lens):
    from concourse.bass_utils import run_bass_kernel_spmd
    tm = tree_bounds_ok(tree_paths)
    if tm not in _NC:
        _NC[tm] = build_bass(tree_mode=tm)
    _nc = _NC[tm]
    table, consts = make_host_tables(class_w, tree_paths, tree_lens,
                                     tree_mode=tm)
    in_maps = make_in_maps(y_pred, y_true, table, consts)
    res = run_bass_kernel_spmd(_nc, in_maps, core_ids=list(range(N_CORES)))
    total = sum(float(r["partial"].sum()) for r in res.results)
    return np.float32(-total / B)


if __name__ == "__main__":
    nc = build_bass()
    print("built OK:", len(nc.m.functions[0].allocations), "allocations")
